# revision 7
# baseline (speedup 1.0000x reference)
"""CRF loss (forward-algorithm logsumexp recurrence) on 8 NeuronCores.

Strategy: data-parallel over batch (B=128 -> 16 per core), with each
core's serial time chain split in half and run from BOTH ENDS
concurrently.  The lattice score 1^T P_{t*} is a chain product, so it
can be evaluated as a forward vector chain from t=0 and a backward
co-state chain from t=TMAX meeting at t=mid:

  fwd:  P_t = (E^T P_{t-1}) * F_t          (exp domain, as baseline)
  bwd:  z_{tau-1} = E (F_tau * (z_tau + 1[tau == t*_b]))
  s_{t*_b} = sum_k z_mid[k,b] * P_mid[k,b]     for t*_b > mid

Both chains are latency-bound serial loops (~520ns per step: PE matmul
transit + DVE multiply + two semaphore hops), so halving the chain
length halves wall-clock while total work is unchanged.  The per-batch
end-point injections ride an extra identity-stationary matmul that
accumulates a host-built indicator stream into the same PSUM group --
data-driven, shared SPMD program, zero critical-path cost.
"""

import numpy as np

B, T, K = 128, 256, 256
N_CORES = 8
BL = B // N_CORES  # batch per core
KT = K // 128      # k tiles (contraction/output splits)
CHUNK = 32         # timesteps of F per DMA chunk
COLS = 2 * BL      # state columns per step: (j, b)

_cache = {}


def _chunk_lens(t):
    # tiny first chunk so step 0 isn't gated on a large DMA
    if t <= 8:
        return [t]
    ls = [4, min(CHUNK, t) - 4]
    rem = t - sum(ls)
    while rem > 0:
        c = min(CHUNK, rem)
        ls.append(c)
        rem -= c
    return ls


class _Stream:
    """Chunked DMA stream of per-step [128, width] column groups."""

    def __init__(self, nc, tc, pool, src_ap, n_steps, name, engines,
                 width=COLS):
        self.nc = nc
        self.pool = pool
        self.src = src_ap
        self.name = name
        self.engines = engines
        self.width = width
        lens = _chunk_lens(n_steps)
        self.t0 = np.cumsum([0] + lens).tolist()
        self.lens = lens
        self.n_chunks = len(lens)
        self.step_chunk = {}
        for ci, (t0c, ln) in enumerate(zip(self.t0, lens)):
            for tt in range(t0c, t0c + ln):
                self.step_chunk[tt] = (ci, tt - t0c)
        self.tiles = [None] * self.n_chunks
        self.next_load = 0

    def load(self, c, split=1, engines=None):
        from concourse import mybir
        engines = engines or self.engines
        ln = self.lens[c]
        ft = self.pool.tile([128, CHUNK * self.width], mybir.dt.float16,
                            tag=self.name, name=self.name)
        w = ln * self.width
        base = self.t0[c] * self.width
        for s in range(split):
            lo, hi = s * w // split, (s + 1) * w // split
            eng = engines[s % len(engines)]
            eng.dma_start(ft[:, lo:hi], self.src[:, base + lo:base + hi])
        self.tiles[c] = ft
        self.next_load = max(self.next_load, c + 1)

    def col(self, t):
        c, r = self.step_chunk[t]
        return self.tiles[c][:, r * self.width:(r + 1) * self.width]

    def maybe_prefetch(self, t, ahead=2):
        c, _ = self.step_chunk[t]
        while self.next_load <= c + ahead and self.next_load < self.n_chunks:
            self.load(self.next_load)


def _build_nc(nfwd, nbwd):
    from contextlib import ExitStack

    import concourse.bacc as bacc
    import concourse.tile as tile
    from concourse import mybir

    nc = bacc.Bacc("TRN2", target_bir_lowering=False, debug=False,
                   enable_asserts=False, num_devices=N_CORES)
    f16 = mybir.dt.float16
    f32 = mybir.dt.float32

    # E tiles: fwd e[i][j] then bwd et[i][j] then identity, packed flat
    NE = KT * KT
    e_in = nc.dram_tensor("e_in", [128, (2 * NE + 1) * 128], f16,
                          kind="ExternalInput").ap()
    f_in = nc.dram_tensor("f_in", [128, nfwd * COLS], f16,
                          kind="ExternalInput").ap()
    # bwd combined stream: slot u = [inj(u) | fb(u)], each COLS wide
    fb_in = nc.dram_tensor("fb_in", [128, (nbwd + 1) * 2 * COLS], f16,
                           kind="ExternalInput").ap()
    s_out = nc.dram_tensor("s_out", [1, nfwd * COLS], f32,
                           kind="ExternalOutput").ap()
    d_out = nc.dram_tensor("d_out", [1, COLS], f32,
                           kind="ExternalOutput").ap()

    WIN = 8          # fwd P' slots per state buffer; s-sum matmul per window
    with tile.TileContext(nc) as tc, ExitStack() as ctx:
        consts = ctx.enter_context(tc.tile_pool(name="consts", bufs=1))
        fpool = ctx.enter_context(tc.tile_pool(name="fpool", bufs=3))
        fbpool = ctx.enter_context(tc.tile_pool(name="fbpool", bufs=3))
        state = ctx.enter_context(tc.tile_pool(name="state", bufs=2))
        vpool = ctx.enter_context(tc.tile_pool(name="vpool", bufs=3))
        psum = ctx.enter_context(tc.tile_pool(name="psum", bufs=2,
                                              space="PSUM"))
        psumz = ctx.enter_context(tc.tile_pool(name="psumz", bufs=2,
                                               space="PSUM"))
        psum_s = ctx.enter_context(tc.tile_pool(name="psum_s", bufs=2,
                                                space="PSUM"))

        fs = _Stream(nc, tc, fpool, f_in, nfwd, "fch", [nc.sync, nc.sync])
        bs = _Stream(nc, tc, fbpool, fb_in, nbwd + 1, "bch",
                     [nc.gpsimd, nc.gpsimd], width=2 * COLS)

        # preamble DMAs: E alone on gpsimd, both tiny chunk-0 loads on
        # sync, so the pre-step's three dependencies land in parallel
        e_all = consts.tile([128, (2 * NE + 1) * 128], f16, tag="eall",
                            name="eall")
        nc.gpsimd.dma_start(e_all[:], e_in[:])
        bs.load(0, engines=[nc.sync])
        fs.load(0, engines=[nc.sync])

        ident = e_all[:, 0:128]
        e_t = [[e_all[:, (1 + i * KT + j) * 128:(2 + i * KT + j) * 128]
                for j in range(KT)] for i in range(KT)]
        et_t = [[e_all[:, (1 + NE + i * KT + j) * 128:
                       (2 + NE + i * KT + j) * 128]
                 for j in range(KT)] for i in range(KT)]
        ones = consts.tile([128, 1], f16, tag="ones", name="ones")
        nc.vector.memset(ones[:], 1.0)

        # warm up the PE p-state during the DMA preamble: ~2us of dummy
        # matmuls so the real chain starts at full clock
        wps = psum_s.tile([1, 1], f32, tag="pss", name="warm")
        for _ in range(24):
            nc.tensor.matmul(wps[:], ones[:], ones[:], start=True, stop=True)

        # s log: one row, all fwd timesteps x (j, b) partials
        s_buf = consts.tile([1, nfwd * COLS], f32, tag="sbuf", name="sbuf")
        d_buf = consts.tile([1, COLS], f32, tag="dbuf", name="dbuf")

        if fs.n_chunks > 1:
            fs.load(1)
        if bs.n_chunks > 1:
            bs.load(1)

        def emit_s(w, pb, lo, ncols):
            # capture cols [lo, lo+ncols) of window w; halves are emitted
            # at different iteration phases so each ~215ns matmul burst
            # fits the PE idle gap instead of displacing the chains
            pss = psum_s.tile([1, WIN * COLS], f32, tag="pss", name="pss")
            nc.tensor.matmul(pss[:, 0:ncols], ones[:], pb[:, lo:lo + ncols],
                             start=True, stop=True)
            base = w * WIN * COLS + lo
            nc.scalar.copy(s_buf[:, base:base + ncols], pss[:, 0:ncols])
            nc.sync.dma_start(s_out[:, base:base + ncols],
                              s_buf[:, base:base + ncols])

        # ---- fwd state: WIN step-slots of COLS in one buffer ----
        pbuf_prev = None
        pending = []
        pbuf = state.tile([128, WIN * COLS], f16, tag="pb", name="pb")
        p_mid = None

        # ---- bwd state: z in PSUM, v in SBUF ----
        # pre-step: psum_z(0) = inj(0)   (injection at tau = TMAX)
        z_ps = None
        if nbwd > 0:
            z_ps = psumz.tile([128, COLS], f32, tag="zps", name="zps")
            nc.tensor.matmul(z_ps[:], ident, bs.col(0)[:, 0:COLS],
                             start=True, stop=True)

        n_loop = max(nfwd, nbwd)
        for i in range(n_loop):
            # ---------------- fwd step t = i ----------------
            if i < nfwd:
                t = i
                fs.maybe_prefetch(t)
                w, slot = divmod(t, WIN)
                fcol = fs.col(t)
                p_new = pbuf[:, slot * COLS:(slot + 1) * COLS]
                if t == 0:
                    nc.vector.tensor_copy(p_new, fcol)
                else:
                    pt = t - 1
                    pslot = pt % WIN
                    src = pbuf_prev if pslot == WIN - 1 else pbuf
                    p_prev = src[:, pslot * COLS:(pslot + 1) * COLS]
                    # start=True zeroes the whole 2KB PSUM zero-region (the
                    # full bank), so only the first matmul starts the group
                    # and both column-half accumulations share the bank
                    ps = psum.tile([128, COLS], f32, tag="ps", name="ps")
                    nc.tensor.matmul(ps[:, 0:BL], e_t[0][0][:],
                                     p_prev[:, 0:BL], start=True, stop=False)
                    nc.tensor.matmul(ps[:, BL:COLS], e_t[0][1][:],
                                     p_prev[:, 0:BL], start=False, stop=False)
                    nc.tensor.matmul(ps[:, 0:BL], e_t[1][0][:],
                                     p_prev[:, BL:COLS], start=False,
                                     stop=False)
                    nc.tensor.matmul(ps[:, BL:COLS], e_t[1][1][:],
                                     p_prev[:, BL:COLS], start=False,
                                     stop=True)
                    nc.vector.tensor_mul(p_new[:], ps[:], fcol[:])
                if t == nfwd - 1:
                    p_mid = p_new
                if slot == WIN - 1 or t == nfwd - 1:
                    for job in pending:     # fwd-only path safety
                        emit_s(*job)
                    pending = []
                    ncols = (slot + 1) * COLS
                    half = (ncols // (2 * COLS)) * COLS
                    if half and ncols - half:
                        pending = [(w, pbuf, 0, half),
                                   (w, pbuf, half, ncols - half)]
                    else:
                        pending = [(w, pbuf, 0, ncols)]
                    pbuf_prev = pbuf
                    pbuf = state.tile([128, WIN * COLS], f16, tag="pb",
                                      name="pb")

            # ---------------- bwd step s = i ----------------
            if i < nbwd:
                s = i
                bs.maybe_prefetch(s)
                # v(s) = fb[s] * z_state(s)
                v = vpool.tile([128, COLS], f16, tag="v", name="v")
                nc.vector.tensor_mul(v[:], z_ps[:], bs.col(s)[:, COLS:])
                # z_state(s+1) = E' v(s) + inj(s+1)
                z_new = psumz.tile([128, COLS], f32, tag="zps", name="zps")
                jcol = bs.col(s + 1)[:, 0:COLS]
                nc.tensor.matmul(z_new[:, 0:BL], ident, jcol[:, 0:BL],
                                 start=True, stop=False)
                nc.tensor.matmul(z_new[:, BL:COLS], ident, jcol[:, BL:COLS],
                                 start=False, stop=False)
                nc.tensor.matmul(z_new[:, 0:BL], et_t[0][0][:],
                                 v[:, 0:BL], start=False, stop=False)
                nc.tensor.matmul(z_new[:, BL:COLS], et_t[0][1][:],
                                 v[:, 0:BL], start=False, stop=False)
                nc.tensor.matmul(z_new[:, 0:BL], et_t[1][0][:],
                                 v[:, BL:COLS], start=False, stop=False)
                nc.tensor.matmul(z_new[:, BL:COLS], et_t[1][1][:],
                                 v[:, BL:COLS], start=False, stop=True)
                z_ps = z_new

            # deferred window captures at end of iteration, one half per
            # emission phase so each burst fits the PE idle gap; once the
            # fwd chain is done, drain during the remaining bwd iterations
            if pending and (i % WIN in (1, 3) or i >= nfwd):
                emit_s(*pending.pop(0))

        for job in pending:
            emit_s(*job)

        # ---- final dot: s_{t*_b} = sum_k z_mid * P_mid  (t* > mid) ----
        if nbwd > 0:
            qv = vpool.tile([128, COLS], f16, tag="v", name="v")
            nc.vector.tensor_mul(qv[:], z_ps[:], p_mid[:])
            psd = psum_s.tile([1, WIN * COLS], f32, tag="pss", name="pss")
            nc.tensor.matmul(psd[:, 0:COLS], ones[:], qv[:], start=True,
                             stop=True)
            nc.scalar.copy(d_buf[:], psd[:, 0:COLS])
            nc.sync.dma_start(d_out[:], d_buf[:])

    nc.compile()
    return nc


def _prepare(feats, transitions, feats_len, nfwd, nbwd, tmax):
    f = np.ascontiguousarray(feats.transpose(1, 0, 2)).astype(np.float32)
    # per-step normalizer: mean over batch of logsumexp_k of the frame
    m = f.max(axis=2)
    lse = np.log(np.exp(f - m[:, :, None]).sum(axis=2,
                                               dtype=np.float32)) + m
    c = lse.mean(axis=1).astype(np.float32)             # [T]
    offs = np.cumsum(c.astype(np.float64))              # [T]

    E = np.exp(transitions.astype(np.float32))
    NE = KT * KT
    e_packed = np.zeros((128, (2 * NE + 1) * 128), np.float16)
    e_packed[:, 0:128] = np.eye(128, dtype=np.float16)
    for i in range(KT):
        for j in range(KT):
            e_packed[:, (1 + i * KT + j) * 128:(2 + i * KT + j) * 128] = \
                E[128 * i:128 * (i + 1), 128 * j:128 * (j + 1)]
            # bwd: out[i_loc, b] += sum_k E[128*ih+i_loc, 128*kh+k] v[k]
            e_packed[:, (1 + NE + i * KT + j) * 128:
                     (2 + NE + i * KT + j) * 128] \
                = E[128 * j:128 * (j + 1), 128 * i:128 * (i + 1)].T

    # F[t, b, k] = exp(f[t, b, k] - c_t)
    Fx = np.exp(f - c[:, None, None]).astype(np.float32)  # [T, B, K]
    tstar = feats_len - 1                                 # [B]

    in_maps = []
    for core in range(N_CORES):
        sl = Fx[:, core * BL:(core + 1) * BL, :]          # [T, BL, K]
        # fwd stream: t ascending 0..nfwd-1
        blk = sl[:nfwd].reshape(nfwd, BL, KT, 128).transpose(3, 0, 2, 1)
        f_map = np.ascontiguousarray(
            blk.reshape(128, nfwd * COLS)).astype(np.float16)
        # bwd combined stream: slot u = [inj(u) | fb(u)]; fb(u) is the
        # frame at tau = tmax - u; inj(u) marks elements with t* == tau,
        # constant across partitions
        comb = np.zeros((128, nbwd + 1, 2 * COLS), np.float32)
        taus = tmax - np.arange(max(nbwd, 1))
        slb = sl[taus]                                    # [nbwd, BL, K]
        blkb = slb.reshape(max(nbwd, 1), BL, KT, 128).transpose(3, 0, 2, 1)
        comb[:, :nbwd, COLS:] = blkb.reshape(128, max(nbwd, 1), COLS)[
            :, :nbwd]
        ts_core = tstar[core * BL:(core + 1) * BL]        # [BL]
        for b in range(BL):
            u = tmax - int(ts_core[b])
            if 0 <= u <= nbwd and int(ts_core[b]) >= nfwd:
                comb[:, u, b] = 1.0
                comb[:, u, BL + b] = 1.0
        fb_map = np.ascontiguousarray(
            comb.reshape(128, (nbwd + 1) * 2 * COLS)).astype(np.float16)
        in_maps.append({"e_in": e_packed, "f_in": f_map, "fb_in": fb_map})
    return in_maps, offs


def _gold_score(feats, transitions, tags, feats_len):
    f = feats.transpose(1, 0, 2).astype(np.float32)       # [T, B, K]
    tg = tags.T.astype(np.int64)                          # [T, B]
    mask = (np.arange(T)[:, None] < feats_len[None, :])
    maskf = mask.astype(np.float32)
    emit = np.take_along_axis(f, tg[:, :, None], axis=2)[:, :, 0] * maskf
    u = emit.sum(axis=0, dtype=np.float32)
    t_mask = maskf[:-1] * maskf[1:]
    t_score = transitions.astype(np.float32)[tg[:-1], tg[1:]] * t_mask
    return (u + t_score.sum(axis=0, dtype=np.float32)).astype(np.float32)


def kernel(feats, transitions, tags, feats_len, _results_hook=None,
           _trace=False):
    from concourse.bass_utils import run_bass_kernel_spmd

    feats = np.asarray(feats, dtype=np.float32)
    transitions = np.asarray(transitions, dtype=np.float32)
    tags_np = np.asarray(tags)
    feats_len_np = np.asarray(feats_len).astype(np.int64)

    tmax = int(feats_len_np.max()) - 1                    # last active step
    # fwd ends 2 iterations before bwd so its final window captures
    # overlap the bwd chain's last steps instead of the tail
    mid = max(tmax // 2 - 2, 0)
    nfwd = mid + 1                                        # fwd covers 0..mid
    nbwd = tmax - mid                                     # bwd covers tmax..mid+1
    if tmax < 24:                                         # degenerate: fwd only
        nfwd, nbwd, mid = tmax + 1, 0, tmax

    if ("nc", nfwd, nbwd) not in _cache:
        _cache[("nc", nfwd, nbwd)] = _build_nc(nfwd, nbwd)
    nc = _cache[("nc", nfwd, nbwd)]

    in_maps, offs = _prepare(feats, transitions, feats_len_np, nfwd, nbwd,
                             tmax)

    res = run_bass_kernel_spmd(nc, in_maps, core_ids=list(range(N_CORES)),
                               trace=_trace)
    if _results_hook is not None:
        _results_hook(res)

    u = _gold_score(feats, transitions, tags_np, feats_len_np)
    loss = np.empty(B, np.float32)
    idx = feats_len_np - 1                                 # [B] capture step
    for core in range(N_CORES):
        s = res.results[core]["s_out"].reshape(nfwd, KT, BL).astype(
            np.float64).sum(axis=1)                        # [nfwd, BL]
        d = res.results[core]["d_out"].reshape(KT, BL).astype(
            np.float64).sum(axis=0)                        # [BL]
        bl = np.arange(BL)
        bg = core * BL + bl
        ts_core = idx[bg]
        sv = np.where(ts_core < nfwd, s[np.minimum(ts_core, nfwd - 1), bl],
                      d)
        loss[bg] = (np.log(sv) + offs[ts_core]).astype(np.float32) - u[bg]
    return loss


# revision 8
# speedup vs baseline: 1.0062x; 1.0062x over previous
"""CRF loss (forward-algorithm logsumexp recurrence) on 8 NeuronCores.

Strategy: data-parallel over batch (B=128 -> 16 per core), with each
core's serial time chain split in half and run from BOTH ENDS
concurrently.  The lattice score 1^T P_{t*} is a chain product, so it
can be evaluated as a forward vector chain from t=0 and a backward
co-state chain from t=TMAX meeting at t=mid:

  fwd:  P_t = (E^T P_{t-1}) * F_t          (exp domain, as baseline)
  bwd:  z_{tau-1} = E (F_tau * (z_tau + 1[tau == t*_b]))
  s_{t*_b} = sum_k z_mid[k,b] * P_mid[k,b]     for t*_b > mid

Both chains are latency-bound serial loops (~520ns per step: PE matmul
transit + DVE multiply + two semaphore hops), so halving the chain
length halves wall-clock while total work is unchanged.  The per-batch
end-point injections ride an extra identity-stationary matmul that
accumulates a host-built indicator stream into the same PSUM group --
data-driven, shared SPMD program, zero critical-path cost.
"""

import numpy as np

B, T, K = 128, 256, 256
N_CORES = 8
BL = B // N_CORES  # batch per core
KT = K // 128      # k tiles (contraction/output splits)
CHUNK = 32         # timesteps of F per DMA chunk
COLS = 2 * BL      # state columns per step: (j, b)

_cache = {}


def _chunk_lens(t):
    # tiny first chunk so step 0 isn't gated on a large DMA
    if t <= 8:
        return [t]
    ls = [4, min(CHUNK, t) - 4]
    rem = t - sum(ls)
    while rem > 0:
        c = min(CHUNK, rem)
        ls.append(c)
        rem -= c
    return ls


class _Stream:
    """Chunked DMA stream of per-step [128, width] column groups."""

    def __init__(self, nc, tc, pool, src_ap, n_steps, name, engines,
                 width=COLS):
        self.nc = nc
        self.pool = pool
        self.src = src_ap
        self.name = name
        self.engines = engines
        self.width = width
        lens = _chunk_lens(n_steps)
        self.t0 = np.cumsum([0] + lens).tolist()
        self.lens = lens
        self.n_chunks = len(lens)
        self.step_chunk = {}
        for ci, (t0c, ln) in enumerate(zip(self.t0, lens)):
            for tt in range(t0c, t0c + ln):
                self.step_chunk[tt] = (ci, tt - t0c)
        self.tiles = [None] * self.n_chunks
        self.next_load = 0

    def load(self, c, split=1, engines=None):
        from concourse import mybir
        engines = engines or self.engines
        ln = self.lens[c]
        ft = self.pool.tile([128, CHUNK * self.width], mybir.dt.float16,
                            tag=self.name, name=self.name)
        w = ln * self.width
        base = self.t0[c] * self.width
        for s in range(split):
            lo, hi = s * w // split, (s + 1) * w // split
            eng = engines[s % len(engines)]
            eng.dma_start(ft[:, lo:hi], self.src[:, base + lo:base + hi])
        self.tiles[c] = ft
        self.next_load = max(self.next_load, c + 1)

    def col(self, t):
        c, r = self.step_chunk[t]
        return self.tiles[c][:, r * self.width:(r + 1) * self.width]

    def maybe_prefetch(self, t, ahead=2):
        c, _ = self.step_chunk[t]
        while self.next_load <= c + ahead and self.next_load < self.n_chunks:
            self.load(self.next_load)


def _build_nc(nfwd, nbwd):
    from contextlib import ExitStack

    import concourse.bacc as bacc
    import concourse.tile as tile
    from concourse import mybir

    nc = bacc.Bacc("TRN2", target_bir_lowering=False, debug=False,
                   enable_asserts=False, num_devices=N_CORES)
    f16 = mybir.dt.float16
    f32 = mybir.dt.float32

    # E tiles: fwd e[i][j] then bwd et[i][j] then identity, packed flat
    NE = KT * KT
    e_in = nc.dram_tensor("e_in", [128, (2 * NE + 1) * 128], f16,
                          kind="ExternalInput").ap()
    f_in = nc.dram_tensor("f_in", [128, nfwd * COLS], f16,
                          kind="ExternalInput").ap()
    # bwd combined stream: slot u = [inj(u) | fb(u)], each COLS wide
    fb_in = nc.dram_tensor("fb_in", [128, (nbwd + 1) * 2 * COLS], f16,
                           kind="ExternalInput").ap()
    s_out = nc.dram_tensor("s_out", [1, nfwd * COLS], f32,
                           kind="ExternalOutput").ap()
    # final dot exported unreduced: host sums over k (higher precision
    # in f64 and a shorter device tail than ones-matmul + copy)
    d_out = nc.dram_tensor("d_out", [128, COLS], f16,
                           kind="ExternalOutput").ap()

    WIN = 8          # fwd P' slots per state buffer; s-sum matmul per window
    with tile.TileContext(nc) as tc, ExitStack() as ctx:
        consts = ctx.enter_context(tc.tile_pool(name="consts", bufs=1))
        fpool = ctx.enter_context(tc.tile_pool(name="fpool", bufs=3))
        fbpool = ctx.enter_context(tc.tile_pool(name="fbpool", bufs=3))
        state = ctx.enter_context(tc.tile_pool(name="state", bufs=2))
        vpool = ctx.enter_context(tc.tile_pool(name="vpool", bufs=3))
        psum = ctx.enter_context(tc.tile_pool(name="psum", bufs=2,
                                              space="PSUM"))
        psumz = ctx.enter_context(tc.tile_pool(name="psumz", bufs=2,
                                               space="PSUM"))
        psum_s = ctx.enter_context(tc.tile_pool(name="psum_s", bufs=2,
                                                space="PSUM"))

        fs = _Stream(nc, tc, fpool, f_in, nfwd, "fch", [nc.sync, nc.sync])
        bs = _Stream(nc, tc, fbpool, fb_in, nbwd + 1, "bch",
                     [nc.gpsimd, nc.gpsimd], width=2 * COLS)

        # preamble DMAs: E alone on gpsimd, both tiny chunk-0 loads on
        # sync, so the pre-step's three dependencies land in parallel
        e_all = consts.tile([128, (2 * NE + 1) * 128], f16, tag="eall",
                            name="eall")
        nc.gpsimd.dma_start(e_all[:], e_in[:])
        bs.load(0, engines=[nc.sync])
        fs.load(0, engines=[nc.sync])

        ident = e_all[:, 0:128]
        e_t = [[e_all[:, (1 + i * KT + j) * 128:(2 + i * KT + j) * 128]
                for j in range(KT)] for i in range(KT)]
        et_t = [[e_all[:, (1 + NE + i * KT + j) * 128:
                       (2 + NE + i * KT + j) * 128]
                 for j in range(KT)] for i in range(KT)]
        ones = consts.tile([128, 1], f16, tag="ones", name="ones")
        nc.vector.memset(ones[:], 1.0)

        # warm up the PE p-state during the DMA preamble: ~2us of dummy
        # matmuls so the real chain starts at full clock
        wps = psum_s.tile([1, 1], f32, tag="pss", name="warm")
        for _ in range(24):
            nc.tensor.matmul(wps[:], ones[:], ones[:], start=True, stop=True)

        # s log: one row, all fwd timesteps x (j, b) partials
        s_buf = consts.tile([1, nfwd * COLS], f32, tag="sbuf", name="sbuf")

        if fs.n_chunks > 1:
            fs.load(1)
        if bs.n_chunks > 1:
            bs.load(1)

        def emit_s(w, pb, lo, ncols):
            # capture cols [lo, lo+ncols) of window w; halves are emitted
            # at different iteration phases so each ~215ns matmul burst
            # fits the PE idle gap instead of displacing the chains
            pss = psum_s.tile([1, WIN * COLS], f32, tag="pss", name="pss")
            nc.tensor.matmul(pss[:, 0:ncols], ones[:], pb[:, lo:lo + ncols],
                             start=True, stop=True)
            base = w * WIN * COLS + lo
            nc.scalar.copy(s_buf[:, base:base + ncols], pss[:, 0:ncols])
            nc.sync.dma_start(s_out[:, base:base + ncols],
                              s_buf[:, base:base + ncols])

        # ---- fwd state: WIN step-slots of COLS in one buffer ----
        pbuf_prev = None
        pending = []
        pbuf = state.tile([128, WIN * COLS], f16, tag="pb", name="pb")
        p_mid = None

        # ---- bwd state: z in PSUM, v in SBUF ----
        # pre-step: psum_z(0) = inj(0)   (injection at tau = TMAX)
        z_ps = None
        if nbwd > 0:
            z_ps = psumz.tile([128, COLS], f32, tag="zps", name="zps")
            nc.tensor.matmul(z_ps[:], ident, bs.col(0)[:, 0:COLS],
                             start=True, stop=True)

        n_loop = max(nfwd, nbwd)
        for i in range(n_loop):
            # ---------------- fwd step t = i ----------------
            if i < nfwd:
                t = i
                fs.maybe_prefetch(t)
                w, slot = divmod(t, WIN)
                fcol = fs.col(t)
                p_new = pbuf[:, slot * COLS:(slot + 1) * COLS]
                if t == 0:
                    nc.vector.tensor_copy(p_new, fcol)
                else:
                    pt = t - 1
                    pslot = pt % WIN
                    src = pbuf_prev if pslot == WIN - 1 else pbuf
                    p_prev = src[:, pslot * COLS:(pslot + 1) * COLS]
                    # start=True zeroes the whole 2KB PSUM zero-region (the
                    # full bank), so only the first matmul starts the group
                    # and both column-half accumulations share the bank
                    ps = psum.tile([128, COLS], f32, tag="ps", name="ps")
                    nc.tensor.matmul(ps[:, 0:BL], e_t[0][0][:],
                                     p_prev[:, 0:BL], start=True, stop=False)
                    nc.tensor.matmul(ps[:, BL:COLS], e_t[0][1][:],
                                     p_prev[:, 0:BL], start=False, stop=False)
                    nc.tensor.matmul(ps[:, 0:BL], e_t[1][0][:],
                                     p_prev[:, BL:COLS], start=False,
                                     stop=False)
                    nc.tensor.matmul(ps[:, BL:COLS], e_t[1][1][:],
                                     p_prev[:, BL:COLS], start=False,
                                     stop=True)
                    nc.vector.tensor_mul(p_new[:], ps[:], fcol[:])
                if t == nfwd - 1:
                    p_mid = p_new
                if slot == WIN - 1 or t == nfwd - 1:
                    for job in pending:     # fwd-only path safety
                        emit_s(*job)
                    pending = []
                    ncols = (slot + 1) * COLS
                    half = (ncols // (2 * COLS)) * COLS
                    if half and ncols - half:
                        pending = [(w, pbuf, 0, half),
                                   (w, pbuf, half, ncols - half)]
                    else:
                        pending = [(w, pbuf, 0, ncols)]
                    pbuf_prev = pbuf
                    pbuf = state.tile([128, WIN * COLS], f16, tag="pb",
                                      name="pb")

            # ---------------- bwd step s = i ----------------
            if i < nbwd:
                s = i
                bs.maybe_prefetch(s)
                # v(s) = fb[s] * z_state(s)
                v = vpool.tile([128, COLS], f16, tag="v", name="v")
                nc.vector.tensor_mul(v[:], z_ps[:], bs.col(s)[:, COLS:])
                # z_state(s+1) = E' v(s) + inj(s+1)
                z_new = psumz.tile([128, COLS], f32, tag="zps", name="zps")
                jcol = bs.col(s + 1)[:, 0:COLS]
                nc.tensor.matmul(z_new[:, 0:BL], ident, jcol[:, 0:BL],
                                 start=True, stop=False)
                nc.tensor.matmul(z_new[:, BL:COLS], ident, jcol[:, BL:COLS],
                                 start=False, stop=False)
                nc.tensor.matmul(z_new[:, 0:BL], et_t[0][0][:],
                                 v[:, 0:BL], start=False, stop=False)
                nc.tensor.matmul(z_new[:, BL:COLS], et_t[0][1][:],
                                 v[:, 0:BL], start=False, stop=False)
                nc.tensor.matmul(z_new[:, 0:BL], et_t[1][0][:],
                                 v[:, BL:COLS], start=False, stop=False)
                nc.tensor.matmul(z_new[:, BL:COLS], et_t[1][1][:],
                                 v[:, BL:COLS], start=False, stop=True)
                z_ps = z_new

            # deferred window captures at end of iteration, one half per
            # emission phase so each burst fits the PE idle gap; once the
            # fwd chain is done, drain during the remaining bwd iterations
            if pending and (i % WIN in (1, 3) or i >= nfwd):
                emit_s(*pending.pop(0))

        for job in pending:
            emit_s(*job)

        # ---- final dot: s_{t*_b} = sum_k z_mid * P_mid  (t* > mid) ----
        if nbwd > 0:
            qv = vpool.tile([128, COLS], f16, tag="v", name="v")
            nc.vector.tensor_mul(qv[:], z_ps[:], p_mid[:])
            nc.sync.dma_start(d_out[:], qv[:])

    nc.compile()
    return nc


def _prepare(feats, transitions, feats_len, nfwd, nbwd, tmax):
    f = np.ascontiguousarray(feats.transpose(1, 0, 2)).astype(np.float32)
    # per-step normalizer: mean over batch of logsumexp_k of the frame
    m = f.max(axis=2)
    lse = np.log(np.exp(f - m[:, :, None]).sum(axis=2,
                                               dtype=np.float32)) + m
    c = lse.mean(axis=1).astype(np.float32)             # [T]
    offs = np.cumsum(c.astype(np.float64))              # [T]

    E = np.exp(transitions.astype(np.float32))
    NE = KT * KT
    e_packed = np.zeros((128, (2 * NE + 1) * 128), np.float16)
    e_packed[:, 0:128] = np.eye(128, dtype=np.float16)
    for i in range(KT):
        for j in range(KT):
            e_packed[:, (1 + i * KT + j) * 128:(2 + i * KT + j) * 128] = \
                E[128 * i:128 * (i + 1), 128 * j:128 * (j + 1)]
            # bwd: out[i_loc, b] += sum_k E[128*ih+i_loc, 128*kh+k] v[k]
            e_packed[:, (1 + NE + i * KT + j) * 128:
                     (2 + NE + i * KT + j) * 128] \
                = E[128 * j:128 * (j + 1), 128 * i:128 * (i + 1)].T

    # F[t, b, k] = exp(f[t, b, k] - c_t)
    Fx = np.exp(f - c[:, None, None]).astype(np.float32)  # [T, B, K]
    tstar = feats_len - 1                                 # [B]

    in_maps = []
    for core in range(N_CORES):
        sl = Fx[:, core * BL:(core + 1) * BL, :]          # [T, BL, K]
        # fwd stream: t ascending 0..nfwd-1
        blk = sl[:nfwd].reshape(nfwd, BL, KT, 128).transpose(3, 0, 2, 1)
        f_map = np.ascontiguousarray(
            blk.reshape(128, nfwd * COLS)).astype(np.float16)
        # bwd combined stream: slot u = [inj(u) | fb(u)]; fb(u) is the
        # frame at tau = tmax - u; inj(u) marks elements with t* == tau,
        # constant across partitions
        comb = np.zeros((128, nbwd + 1, 2 * COLS), np.float32)
        taus = tmax - np.arange(max(nbwd, 1))
        slb = sl[taus]                                    # [nbwd, BL, K]
        blkb = slb.reshape(max(nbwd, 1), BL, KT, 128).transpose(3, 0, 2, 1)
        comb[:, :nbwd, COLS:] = blkb.reshape(128, max(nbwd, 1), COLS)[
            :, :nbwd]
        ts_core = tstar[core * BL:(core + 1) * BL]        # [BL]
        for b in range(BL):
            u = tmax - int(ts_core[b])
            if 0 <= u <= nbwd and int(ts_core[b]) >= nfwd:
                comb[:, u, b] = 1.0
                comb[:, u, BL + b] = 1.0
        fb_map = np.ascontiguousarray(
            comb.reshape(128, (nbwd + 1) * 2 * COLS)).astype(np.float16)
        in_maps.append({"e_in": e_packed, "f_in": f_map, "fb_in": fb_map})
    return in_maps, offs


def _gold_score(feats, transitions, tags, feats_len):
    f = feats.transpose(1, 0, 2).astype(np.float32)       # [T, B, K]
    tg = tags.T.astype(np.int64)                          # [T, B]
    mask = (np.arange(T)[:, None] < feats_len[None, :])
    maskf = mask.astype(np.float32)
    emit = np.take_along_axis(f, tg[:, :, None], axis=2)[:, :, 0] * maskf
    u = emit.sum(axis=0, dtype=np.float32)
    t_mask = maskf[:-1] * maskf[1:]
    t_score = transitions.astype(np.float32)[tg[:-1], tg[1:]] * t_mask
    return (u + t_score.sum(axis=0, dtype=np.float32)).astype(np.float32)


def kernel(feats, transitions, tags, feats_len, _results_hook=None,
           _trace=False):
    from concourse.bass_utils import run_bass_kernel_spmd

    feats = np.asarray(feats, dtype=np.float32)
    transitions = np.asarray(transitions, dtype=np.float32)
    tags_np = np.asarray(tags)
    feats_len_np = np.asarray(feats_len).astype(np.int64)

    tmax = int(feats_len_np.max()) - 1                    # last active step
    # fwd ends 2 iterations before bwd so its final window captures
    # overlap the bwd chain's last steps instead of the tail
    mid = max(tmax // 2 - 2, 0)
    nfwd = mid + 1                                        # fwd covers 0..mid
    nbwd = tmax - mid                                     # bwd covers tmax..mid+1
    if tmax < 24:                                         # degenerate: fwd only
        nfwd, nbwd, mid = tmax + 1, 0, tmax

    if ("nc", nfwd, nbwd) not in _cache:
        _cache[("nc", nfwd, nbwd)] = _build_nc(nfwd, nbwd)
    nc = _cache[("nc", nfwd, nbwd)]

    in_maps, offs = _prepare(feats, transitions, feats_len_np, nfwd, nbwd,
                             tmax)

    res = run_bass_kernel_spmd(nc, in_maps, core_ids=list(range(N_CORES)),
                               trace=_trace)
    if _results_hook is not None:
        _results_hook(res)

    u = _gold_score(feats, transitions, tags_np, feats_len_np)
    loss = np.empty(B, np.float32)
    idx = feats_len_np - 1                                 # [B] capture step
    for core in range(N_CORES):
        s = res.results[core]["s_out"].reshape(nfwd, KT, BL).astype(
            np.float64).sum(axis=1)                        # [nfwd, BL]
        d = res.results[core]["d_out"].reshape(128, KT, BL).astype(
            np.float64).sum(axis=(0, 1))                   # [BL]
        bl = np.arange(BL)
        bg = core * BL + bl
        ts_core = idx[bg]
        sv = np.where(ts_core < nfwd, s[np.minimum(ts_core, nfwd - 1), bl],
                      d)
        loss[bg] = (np.log(sv) + offs[ts_core]).astype(np.float32) - u[bg]
    return loss
